# revision 38
# baseline (speedup 1.0000x reference)
"""Trainium2 Bass kernel for GQA attention block (B=2, S=2048, H=2048,
16 q-heads / 4 kv-heads, head_dim=128, RoPE, causal) on 8 NeuronCores.

Sharding: core c -> batch b = c // 4, kv-group g = c % 4
  (q heads 4g..4g+3, kv head g).  Each core computes its batch's
  attention for its 4 query heads plus the partial output projection
  over its 512 hidden columns of w_o; host sums the 4 partials per batch.

v3: bf16 end-to-end (halves HBM traffic + DVE cost; fp32 PSUM accumulate
keeps rel err ~1e-3).  QKV projection iterates kt-outer across 6
parallel PSUM chains so each weight/activation slab is consumed as it
lands (PE starts ~2us after launch).  DMA loads are batched and split
across the SP and Activation HWDGE queues.  RoPE rotate-half is done
with partition-offset DVE ops (no SBUF->SBUF DMAs).  The softmax
denominator is accumulated on DVE (e-tile running sum) and reduced
across partitions with one GpSimd partition_all_reduce per (qb, head),
freeing ~29us of PE ones-matmul time.  Causal mask add narrowed to the
128-wide diagonal sub-block.

On-chip layouts (per core):
  qT/kT    [head_dim=128 part, S free] bf16  (projection emits transposed)
  v        [S part-blocks, head_dim] bf16    (PE transpose of vT; PV lhsT)
  scoresT  [sk part, sq free] f32 PSUM -> exp -> bf16 e
  PV accumulates out^T [d, sq] in PSUM over j
  o-proj emits out[s, o] bf16; host upcasts + sums 4 partials per batch
"""

import contextlib
import math
import numpy as np

import concourse.bacc as bacc
import concourse.bass_isa as bass_isa
import concourse.mybir as mybir
import concourse.tile as tile
from concourse.bass_utils import run_bass_kernel_spmd
from concourse.masks import make_identity

F32 = mybir.dt.float32
BF16 = mybir.dt.bfloat16
AF = mybir.ActivationFunctionType
NPBF16 = mybir.dt.np(BF16)

S = 2048
H = 2048
D = 128            # head dim
KT = 16            # contraction tiles over hidden (2048/128)
NQ = 512           # query block width in attention
NUM_Q_LOCAL = 4    # q heads per core
SCALE = 1.0 / math.sqrt(D)
NEG = -1.0e9

_CACHED = {}


class _SkipExc(Exception):
    pass


class _Skip:
    """Context manager that skips its with-body entirely."""

    def __enter__(self):
        import sys
        import inspect
        self._tr = sys.gettrace()
        sys.settrace(lambda *a, **k: None)
        frame = inspect.currentframe().f_back
        frame.f_trace = self._trace
        return self

    def _trace(self, frame, event, arg):
        raise _SkipExc

    def __exit__(self, exc_type, exc, tb):
        import sys
        sys.settrace(self._tr)
        return exc_type is _SkipExc


def build_nc(mm_dt=BF16, loop_n=None, phases=3):
    nc = bacc.Bacc(None, target_bir_lowering=False)
    # host-packed layouts (see _prep_inputs):
    #   hTp  [128 p, 16 kt, 2048 s]   hidden[b].T, p = h % 128, kt = h // 128
    #   wqk  [128 p, 16 kt, 6 i, 128 m]
    #   wo   [128 d, 4 kb, 2048 o]
    hTp = nc.dram_tensor("hTp", [128, 4, KT, 512], mm_dt, kind="ExternalInput")
    wqk = nc.dram_tensor("wqk", [128, KT, 6, 128], mm_dt, kind="ExternalInput")
    cosT = nc.dram_tensor("cosT", [D, S], mm_dt, kind="ExternalInput")
    sinT = nc.dram_tensor("sinT", [D, S], mm_dt, kind="ExternalInput")
    tri = nc.dram_tensor("tri", [128, 128], mm_dt, kind="ExternalInput")
    wo = nc.dram_tensor("wo", [128, 4, H], mm_dt, kind="ExternalInput")
    out = nc.dram_tensor("out", [S, H], mm_dt, kind="ExternalOutput")

    wqk_flat = wqk.reshape([128, KT * 6 * 128])
    wo_flat = wo.reshape([128, 4 * H])

    with tile.TileContext(nc) as tc:
        with tc.tile_pool(name="persist", bufs=1) as pp:
          with (tc.For_i(0, loop_n, 1) if loop_n else contextlib.nullcontext()):
            # ---- persistent tiles (live across phases) ----
            qk = [pp.tile([128, S], mm_dt, name=f"qk{i}", tag=f"qk{i}") for i in range(5)]
            v_sb = pp.tile([128, S], mm_dt, tag="v")
            ones_r = pp.tile([128, 1], mm_dt, tag="onesr")
            cos_sb = pp.tile([128, S], mm_dt, tag="cos")
            sin_sb = pp.tile([128, S], mm_dt, tag="sin")
            ident = pp.tile([128, 128], mm_dt, tag="ident")
            tri_sb = pp.tile([128, 128], mm_dt, tag="tri")
            w_sb = pp.tile([128, KT * 6 * 128], mm_dt, tag="wsb")
            wo_sb = pp.tile([128, 4 * H], mm_dt, tag="wosb")

            # ---- Phase 1: fused QKV projection, kt-outer over 6 PSUM chains
            # + RoPE and v-transpose per quarter ----
            QW = 512
            NQT = S // QW
            with (
                tc.tile_pool(name="ht", bufs=2) as htp,
                tc.tile_pool(name="vtp", bufs=1) as vtp,
                tc.tile_pool(name="rope", bufs=3) as rp,
                tc.tile_pool(name="psq", bufs=1, space="PSUM") as psq,
                tc.tile_pool(name="psv", bufs=2, space="PSUM") as psv,
            ):
                vT_sb = vtp.tile([128, S], mm_dt, tag="vT")
                ht = [None] * NQT

                def load_ht(q, split=False):
                    # per-partition contiguous runs: 4KB (split groups) or
                    # 16KB (whole quarter)
                    ht[q] = htp.tile(
                        [128, KT, QW], mm_dt, name=f"htq{q}", tag=f"ht{q % 2}"
                    )
                    if split:
                        for g4 in range(4):
                            nc.sync.dma_start(
                                out=ht[q][:, g4 * 4 : (g4 + 1) * 4, :],
                                in_=hTp[:, q, g4 * 4 : (g4 + 1) * 4, :],
                            )
                    else:
                        nc.sync.dma_start(out=ht[q][:], in_=hTp[:, q])

                # first weight slab + first ht group lead their rings so the
                # first matmul starts ~2us in; graduated slab sizes after
                ht[0] = htp.tile([128, KT, QW], mm_dt, name="htq0", tag="ht0")
                nc.scalar.dma_start(
                    out=w_sb[:, 0:768], in_=wqk_flat[:, 0:768]
                )
                nc.sync.dma_start(out=ht[0][:, 0:4, :], in_=hTp[:, 0, 0:4, :])
                nc.scalar.dma_start(
                    out=w_sb[:, 768 : 4 * 768], in_=wqk_flat[:, 768 : 4 * 768]
                )
                nc.sync.dma_start(out=ht[0][:, 4:16, :], in_=hTp[:, 0, 4:16, :])
                for half in range(2):
                    sl = slice((4 + 6 * half) * 768, (10 + 6 * half) * 768)
                    nc.scalar.dma_start(out=w_sb[:, sl], in_=wqk_flat[:, sl])
                nc.scalar.dma_start(out=cos_sb[:], in_=cosT[:])
                nc.scalar.dma_start(out=sin_sb[:], in_=sinT[:])
                nc.scalar.dma_start(out=tri_sb[:], in_=tri[:])
                nc.scalar.dma_start(out=wo_sb[:], in_=wo_flat[:])
                nc.vector.memset(ones_r[:], 1.0)
                make_identity(nc, ident[:])

                for q in range(NQT):
                    s0 = q * QW
                    if q + 1 < NQT:
                        load_ht(q + 1)
                    ps = [
                        psq.tile([128, QW], F32, name=f"ps{i}", tag=f"psq{i}")
                        for i in range(6)
                    ]
                    for kt in range(KT):
                        for i in range(6):
                            nc.tensor.matmul(
                                ps[i][:],
                                lhsT=w_sb[:, (kt * 6 + i) * 128 : (kt * 6 + i + 1) * 128],
                                rhs=ht[q][:, kt, :],
                                start=(kt == 0),
                                stop=(kt == KT - 1),
                            )
                    for i in range(6):
                        if i < 5:
                            # RoPE: qk = raw*cos + rot_half(raw)*sin
                            # (sin rows 0:64 pre-negated host-side)
                            # SB+SB tensor ops need equal INPUT base
                            # partitions; sin is stored half-swapped
                            # host-side so the rotate-half muls read raw and
                            # sin at the same base and only the OUTPUT is
                            # partition-shifted.
                            raw = rp.tile([128, QW], mm_dt, tag="raw")
                            if i % 2 == 0:
                                nc.scalar.activation(raw[:], ps[i][:], AF.Copy)
                            else:
                                nc.vector.tensor_copy(raw[:], ps[i][:])
                            t1 = rp.tile([128, QW], mm_dt, tag="t1")
                            t2 = rp.tile([128, QW], mm_dt, tag="t2")
                            nc.vector.tensor_mul(
                                t1[64:128, :], raw[0:64, :], sin_sb[0:64, s0 : s0 + QW]
                            )
                            nc.vector.tensor_mul(
                                t1[0:64, :], raw[64:128, :], sin_sb[64:128, s0 : s0 + QW]
                            )
                            nc.vector.tensor_mul(
                                t2[:], raw[:], cos_sb[:, s0 : s0 + QW]
                            )
                            nc.vector.tensor_add(
                                qk[i][:, s0 : s0 + QW], t1[:], t2[:]
                            )
                        else:
                            nc.scalar.activation(
                                vT_sb[:, s0 : s0 + QW], ps[i][:], AF.Copy
                            )
                            for sbl in range(QW // 128):
                                sb = q * (QW // 128) + sbl
                                psvt = psv.tile([128, 128], mm_dt, tag="psv")
                                nc.tensor.transpose(
                                    psvt[:],
                                    vT_sb[:, sb * 128 : (sb + 1) * 128],
                                    ident[:],
                                )
                                nc.scalar.activation(
                                    v_sb[:, sb * 128 : (sb + 1) * 128], psvt[:],
                                    AF.Copy,
                                )

            # ---- Phase 2 + 3 interleaved per query block ----
            with (
                contextlib.nullcontext() if phases >= 2 else _Skip(),
                tc.tile_pool(name="attn", bufs=1) as ap,
                tc.tile_pool(name="epool", bufs=8) as ep,
                tc.tile_pool(name="accp", bufs=2) as accp,
                tc.tile_pool(name="small", bufs=2) as sp,
                tc.tile_pool(name="osb", bufs=2) as op,
                tc.tile_pool(name="pss", bufs=2, space="PSUM") as pss,
                tc.tile_pool(name="pspv", bufs=1, space="PSUM") as pspv,
                tc.tile_pool(name="pso", bufs=2, space="PSUM") as pso,
            ):
                attnT = [
                    ap.tile([128, S], mm_dt, name=f"at{h}", tag=f"at{h}")
                    for h in range(4)
                ]

                kT = qk[4]

                # o-projection is deferred one query block and interleaved
                # into the next block's chain loops: PE does o-proj matmuls
                # while ACT grinds the exps (they were serializing before).
                pending = []
                osb_box = [None]

                def oproj_unit(sb, n):
                    def emit():
                        if n == 0:
                            osb_box[0] = op.tile(
                                [128, H], mm_dt, name=f"osb{sb}", tag="osb"
                            )
                        osb = osb_box[0]
                        pst = pso.tile([128, NQ], F32, tag="po")
                        for kb in range(4):
                            nc.tensor.matmul(
                                pst[:],
                                lhsT=attnT[kb][:, sb * 128 : (sb + 1) * 128],
                                rhs=wo_sb[:, kb * H + n * NQ : kb * H + (n + 1) * NQ],
                                start=(kb == 0),
                                stop=(kb == 3),
                            )
                        nc.vector.tensor_copy(osb[:, n * NQ : (n + 1) * NQ], pst[:])
                        if n == 3:
                            nc.sync.dma_start(
                                out=out[sb * 128 : (sb + 1) * 128, :], in_=osb[:]
                            )
                    return emit

                def flush_oproj(k):
                    for _ in range(k):
                        if pending:
                            pending.pop(0)()

                for qb in range(S // NQ):
                    q0 = qb * NQ
                    nj = 4 * qb + 4
                    # the two heads of a pair share one 2-bank scores tile
                    # ([128, 2*NQ]), so each j-step is 2 score mms -> ONE
                    # wide exp -> 2 pv mms -> ONE wide acc add: ~half the
                    # instructions/semaphores of per-head emission, and the
                    # per-step chain latency is amortized over 1024 columns
                    for pair in range(2):
                        h0, h1 = 2 * pair, 2 * pair + 1
                        pv = {
                            h: pspv.tile(
                                [128, NQ], F32, name=f"pv{h}", tag=f"pv{h % 2}"
                            )
                            for h in (h0, h1)
                        }
                        acc = accp.tile([128, 2 * NQ], mm_dt, tag="acc")
                        for j in range(nj):
                            r4 = j - 4 * qb
                            # diagonal blocks: columns sq < r4*128 are fully
                            # masked -> narrow the whole j-chain to [off:NQ)
                            off = max(0, r4) * 128
                            sps = pss.tile([128, 2 * NQ], F32, tag="sc")
                            for idx, h in enumerate((h0, h1)):
                                nc.tensor.matmul(
                                    sps[:, idx * NQ + off : (idx + 1) * NQ],
                                    lhsT=kT[:, j * 128 : (j + 1) * 128],
                                    rhs=qk[h][:, q0 + off : q0 + NQ],
                                    start=True,
                                    stop=True,
                                )
                            e = ep.tile([128, 2 * NQ], mm_dt, tag="e")
                            if off == 0:
                                nc.scalar.activation(
                                    e[:], sps[:], AF.Exp, scale=SCALE
                                )
                            else:
                                for idx in range(2):
                                    nc.scalar.activation(
                                        e[:, idx * NQ + off : (idx + 1) * NQ],
                                        sps[:, idx * NQ + off : (idx + 1) * NQ],
                                        AF.Exp, scale=SCALE,
                                    )
                            if r4 >= 0:
                                # causal mask as post-exp 0/1 multiply on the
                                # 128 diagonal cols of each half
                                for idx in range(2):
                                    nc.vector.tensor_mul(
                                        e[:, idx * NQ + off : idx * NQ + off + 128],
                                        e[:, idx * NQ + off : idx * NQ + off + 128],
                                        tri_sb[:],
                                    )
                            for idx, h in enumerate((h0, h1)):
                                nc.tensor.matmul(
                                    pv[h][:, off:NQ],
                                    lhsT=v_sb[:, j * 128 : (j + 1) * 128],
                                    rhs=e[:, idx * NQ + off : (idx + 1) * NQ],
                                    start=(j == 0),
                                    stop=(j == nj - 1),
                                )
                            # softmax denominator: running sum of e on DVE
                            if j == 0:
                                nc.vector.tensor_copy(acc[:], e[:])
                            elif off == 0:
                                nc.vector.tensor_add(acc[:], acc[:], e[:])
                            else:
                                for idx in range(2):
                                    nc.vector.tensor_add(
                                        acc[:, idx * NQ + off : (idx + 1) * NQ],
                                        acc[:, idx * NQ + off : (idx + 1) * NQ],
                                        e[:, idx * NQ + off : (idx + 1) * NQ],
                                    )
                            # one deferred o-proj chain per j-step keeps the
                            # PE busy while ACT runs this step's exp
                            flush_oproj(1)
                        # partition-reduce the pair's acc on GpSimd (output
                        # arrives already broadcast across partitions)
                        den = sp.tile([128, 2 * NQ], F32, tag="den")
                        nc.gpsimd.partition_all_reduce(
                            den[:], acc[:], channels=128,
                            reduce_op=bass_isa.ReduceOp.add,
                        )
                        rec = sp.tile([128, 2 * NQ], F32, tag="rec")
                        nc.vector.reciprocal(rec[:], den[:])
                        for idx, h in enumerate((h0, h1)):
                            nc.vector.tensor_mul(
                                attnT[h][:, q0 : q0 + NQ], pv[h][:],
                                rec[:, idx * NQ : (idx + 1) * NQ],
                            )
                    # queue this query block's o-projection for interleaved
                    # emission during the next block's chains
                    if phases >= 3:
                        for sbl in range(NQ // 128):
                            sb = qb * 4 + sbl
                            for n in range(H // NQ):
                                pending.append(oproj_unit(sb, n))
                flush_oproj(len(pending))

    nc.compile()
    return nc


def _prep_inputs(hidden_states, cos, sin, w_qkv, w_o):
    """Build the 8 per-core input maps (host-side shard + transpose, bf16)."""
    hidden_states = np.asarray(hidden_states, dtype=np.float32)
    cos = np.asarray(cos, dtype=np.float32)
    sin = np.asarray(sin, dtype=np.float32)
    w_qkv = np.asarray(w_qkv, dtype=np.float32)
    w_o = np.asarray(w_o, dtype=np.float32)

    cosT = np.ascontiguousarray(cos.T).astype(NPBF16)
    # sin table half-swapped with rotate_half sign folded in:
    #   rows 0:64  hold  sin.T[64:128]  (multiplies raw[0:64] -> t1[64:128])
    #   rows 64:128 hold -sin.T[0:64]   (multiplies raw[64:128] -> t1[0:64])
    sinTf = np.ascontiguousarray(sin.T)
    sinT = np.concatenate([sinTf[64:128], -sinTf[0:64]]).astype(NPBF16)

    # tri[sk, sq] = 1 if sk <= sq else 0  (post-exp diagonal-block mask)
    sk = np.arange(128)[:, None]
    sq = np.arange(128)[None, :]
    tri = np.where(sk <= sq, 1.0, 0.0).astype(NPBF16)

    # hTp[p, q, kt, s'] = hidden[b].T[kt*128+p, q*512+s']
    hTp = [
        np.ascontiguousarray(
            hidden_states[b].T.reshape(KT, 128, 4, 512).transpose(1, 2, 0, 3)
        ).astype(NPBF16)
        for b in range(2)
    ]

    in_maps = []
    for c in range(8):
        b, g = divmod(c, 4)
        W6 = np.stack(
            [w_qkv[(4 * g + i) * 128 : (4 * g + i + 1) * 128] for i in range(4)]
            + [w_qkv[(16 + g) * 128 : (17 + g) * 128]]
            + [w_qkv[(20 + g) * 128 : (21 + g) * 128]]
        )  # [6 i, 128 m, 2048 h]
        # wqk_pack[p, kt, i, m] = W6[i, m, kt*128+p]
        wqk_pack = np.ascontiguousarray(
            W6.transpose(2, 0, 1).reshape(KT, 128, 6, 128).transpose(1, 0, 2, 3)
        ).astype(NPBF16)
        # wo_pack[d, kb, o] = w_o[o, (4g+kb)*128+d]
        wo_pack = np.ascontiguousarray(
            w_o[:, 4 * g * 128 : (4 * g + 4) * 128]
            .T.reshape(4, 128, H)
            .transpose(1, 0, 2)
        ).astype(NPBF16)
        in_maps.append(
            dict(
                hTp=hTp[b],
                wqk=wqk_pack,
                cosT=cosT,
                sinT=sinT,
                tri=tri,
                wo=wo_pack,
            )
        )
    return in_maps


def run(hidden_states, cos, sin, w_qkv, w_o, trace=False, **trace_kwargs):
    if "nc" not in _CACHED:
        _CACHED["nc"] = build_nc()
    nc = _CACHED["nc"]
    in_maps = _prep_inputs(hidden_states, cos, sin, w_qkv, w_o)
    res = run_bass_kernel_spmd(
        nc, in_maps, core_ids=list(range(8)), trace=trace, **trace_kwargs
    )
    outs = [res.results[c]["out"].astype(np.float32) for c in range(8)]
    full = np.stack(
        [
            outs[0] + outs[1] + outs[2] + outs[3],
            outs[4] + outs[5] + outs[6] + outs[7],
        ]
    ).astype(np.float32)
    return full, res


def kernel(hidden_states, cos, sin, w_qkv, w_o):
    full, _ = run(hidden_states, cos, sin, w_qkv, w_o, trace=False)
    return full


# revision 40
# speedup vs baseline: 1.1654x; 1.1654x over previous
"""Trainium2 Bass kernel for GQA attention block (B=2, S=2048, H=2048,
16 q-heads / 4 kv-heads, head_dim=128, RoPE, causal) on 8 NeuronCores.

Sharding: core c -> batch b = c // 4, kv-group g = c % 4
  (q heads 4g..4g+3, kv head g).  Each core computes its batch's
  attention for its 4 query heads plus the partial output projection
  over its 512 hidden columns of w_o; host sums the 4 partials per batch.

v3: bf16 end-to-end (halves HBM traffic + DVE cost; fp32 PSUM accumulate
keeps rel err ~1e-3).  QKV projection iterates kt-outer across 6
parallel PSUM chains so each weight/activation slab is consumed as it
lands (PE starts ~2us after launch).  DMA loads are batched and split
across the SP and Activation HWDGE queues.  RoPE rotate-half is done
with partition-offset DVE ops (no SBUF->SBUF DMAs).  The softmax
denominator is accumulated on DVE (e-tile running sum) and reduced
across partitions with one GpSimd partition_all_reduce per (qb, head),
freeing ~29us of PE ones-matmul time.  Causal mask add narrowed to the
128-wide diagonal sub-block.

On-chip layouts (per core):
  qT/kT    [head_dim=128 part, S free] bf16  (projection emits transposed)
  v        [S part-blocks, head_dim] bf16    (PE transpose of vT; PV lhsT)
  scoresT  [sk part, sq free] f32 PSUM -> exp -> bf16 e
  PV accumulates out^T [d, sq] in PSUM over j
  o-proj emits out[s, o] bf16; host upcasts + sums 4 partials per batch
"""

import contextlib
import math
import numpy as np

import concourse.bacc as bacc
import concourse.bass_isa as bass_isa
import concourse.mybir as mybir
import concourse.tile as tile
from concourse.bass_utils import run_bass_kernel_spmd
from concourse.masks import make_identity

F32 = mybir.dt.float32
BF16 = mybir.dt.bfloat16
AF = mybir.ActivationFunctionType
NPBF16 = mybir.dt.np(BF16)

S = 2048
H = 2048
D = 128            # head dim
KT = 16            # contraction tiles over hidden (2048/128)
NQ = 512           # query block width in attention
NUM_Q_LOCAL = 4    # q heads per core
SCALE = 1.0 / math.sqrt(D)
NEG = -1.0e9

_CACHED = {}


class _SkipExc(Exception):
    pass


class _Skip:
    """Context manager that skips its with-body entirely."""

    def __enter__(self):
        import sys
        import inspect
        self._tr = sys.gettrace()
        sys.settrace(lambda *a, **k: None)
        frame = inspect.currentframe().f_back
        frame.f_trace = self._trace
        return self

    def _trace(self, frame, event, arg):
        raise _SkipExc

    def __exit__(self, exc_type, exc, tb):
        import sys
        sys.settrace(self._tr)
        return exc_type is _SkipExc


def build_nc(mm_dt=BF16, loop_n=None, phases=3):
    nc = bacc.Bacc(None, target_bir_lowering=False)
    # host-packed layouts (see _prep_inputs):
    #   hTp  [128 p, 16 kt, 2048 s]   hidden[b].T, p = h % 128, kt = h // 128
    #   wqk  [128 p, 16 kt, 6 i, 128 m]
    #   wo   [128 d, 4 kb, 2048 o]
    hTp = nc.dram_tensor("hTp", [128, 4, KT, 512], mm_dt, kind="ExternalInput")
    wqk = nc.dram_tensor("wqk", [128, KT, 6, 128], mm_dt, kind="ExternalInput")
    cosT = nc.dram_tensor("cosT", [D, S], mm_dt, kind="ExternalInput")
    sinT = nc.dram_tensor("sinT", [D, S], mm_dt, kind="ExternalInput")
    tri = nc.dram_tensor("tri", [128, 128], mm_dt, kind="ExternalInput")
    wo = nc.dram_tensor("wo", [128, 4, H], mm_dt, kind="ExternalInput")
    out = nc.dram_tensor("out", [S, H], mm_dt, kind="ExternalOutput")

    wqk_flat = wqk.reshape([128, KT * 6 * 128])
    wo_flat = wo.reshape([128, 4 * H])

    with tile.TileContext(nc) as tc:
        with tc.tile_pool(name="persist", bufs=1) as pp:
          with (tc.For_i(0, loop_n, 1) if loop_n else contextlib.nullcontext()):
            # ---- persistent tiles (live across phases) ----
            qk = [pp.tile([128, S], mm_dt, name=f"qk{i}", tag=f"qk{i}") for i in range(5)]
            v_sb = pp.tile([128, S], mm_dt, tag="v")
            ones_r = pp.tile([128, 1], mm_dt, tag="onesr")
            cos_sb = pp.tile([128, S], mm_dt, tag="cos")
            sin_sb = pp.tile([128, S], mm_dt, tag="sin")
            ident = pp.tile([128, 128], mm_dt, tag="ident")
            tri_sb = pp.tile([128, 128], mm_dt, tag="tri")
            w_sb = pp.tile([128, KT * 6 * 128], mm_dt, tag="wsb")
            wo_sb = pp.tile([128, 4 * H], mm_dt, tag="wosb")

            # ---- Phase 1: fused QKV projection, kt-outer over 6 PSUM chains
            # + RoPE and v-transpose per quarter ----
            QW = 512
            NQT = S // QW
            with (
                tc.tile_pool(name="ht", bufs=2) as htp,
                tc.tile_pool(name="vtp", bufs=1) as vtp,
                tc.tile_pool(name="rope", bufs=3) as rp,
                tc.tile_pool(name="psq", bufs=1, space="PSUM") as psq,
                tc.tile_pool(name="psv", bufs=2, space="PSUM") as psv,
            ):
                vT_sb = vtp.tile([128, S], mm_dt, tag="vT")
                ht = [None] * NQT

                def load_ht(q, split=False):
                    # per-partition contiguous runs: 4KB (split groups) or
                    # 16KB (whole quarter)
                    ht[q] = htp.tile(
                        [128, KT, QW], mm_dt, name=f"htq{q}", tag=f"ht{q % 2}"
                    )
                    if split:
                        for g4 in range(4):
                            nc.sync.dma_start(
                                out=ht[q][:, g4 * 4 : (g4 + 1) * 4, :],
                                in_=hTp[:, q, g4 * 4 : (g4 + 1) * 4, :],
                            )
                    else:
                        nc.sync.dma_start(out=ht[q][:], in_=hTp[:, q])

                # first weight slab + first ht group lead their rings so the
                # first matmul starts ~2us in; graduated slab sizes after
                ht[0] = htp.tile([128, KT, QW], mm_dt, name="htq0", tag="ht0")
                nc.scalar.dma_start(
                    out=w_sb[:, 0:768], in_=wqk_flat[:, 0:768]
                )
                nc.sync.dma_start(out=ht[0][:, 0:4, :], in_=hTp[:, 0, 0:4, :])
                nc.scalar.dma_start(
                    out=w_sb[:, 768 : 4 * 768], in_=wqk_flat[:, 768 : 4 * 768]
                )
                nc.sync.dma_start(out=ht[0][:, 4:16, :], in_=hTp[:, 0, 4:16, :])
                for half in range(2):
                    sl = slice((4 + 6 * half) * 768, (10 + 6 * half) * 768)
                    nc.scalar.dma_start(out=w_sb[:, sl], in_=wqk_flat[:, sl])
                nc.scalar.dma_start(out=cos_sb[:], in_=cosT[:])
                nc.scalar.dma_start(out=sin_sb[:], in_=sinT[:])
                nc.scalar.dma_start(out=tri_sb[:], in_=tri[:])
                nc.scalar.dma_start(out=wo_sb[:], in_=wo_flat[:])
                nc.vector.memset(ones_r[:], 1.0)
                make_identity(nc, ident[:])

                for q in range(NQT):
                    s0 = q * QW
                    if q + 1 < NQT:
                        load_ht(q + 1)
                    ps = [
                        psq.tile([128, QW], F32, name=f"ps{i}", tag=f"psq{i}")
                        for i in range(6)
                    ]
                    for kt in range(KT):
                        for i in range(6):
                            nc.tensor.matmul(
                                ps[i][:],
                                lhsT=w_sb[:, (kt * 6 + i) * 128 : (kt * 6 + i + 1) * 128],
                                rhs=ht[q][:, kt, :],
                                start=(kt == 0),
                                stop=(kt == KT - 1),
                            )
                    for i in range(6):
                        if i < 5:
                            # RoPE: qk = raw*cos + rot_half(raw)*sin
                            # (sin rows 0:64 pre-negated host-side)
                            # SB+SB tensor ops need equal INPUT base
                            # partitions; sin is stored half-swapped
                            # host-side so the rotate-half muls read raw and
                            # sin at the same base and only the OUTPUT is
                            # partition-shifted.
                            raw = rp.tile([128, QW], mm_dt, tag="raw")
                            if i % 2 == 0:
                                nc.scalar.activation(raw[:], ps[i][:], AF.Copy)
                            else:
                                nc.vector.tensor_copy(raw[:], ps[i][:])
                            t1 = rp.tile([128, QW], mm_dt, tag="t1")
                            t2 = rp.tile([128, QW], mm_dt, tag="t2")
                            nc.vector.tensor_mul(
                                t1[64:128, :], raw[0:64, :], sin_sb[0:64, s0 : s0 + QW]
                            )
                            nc.vector.tensor_mul(
                                t1[0:64, :], raw[64:128, :], sin_sb[64:128, s0 : s0 + QW]
                            )
                            nc.vector.tensor_mul(
                                t2[:], raw[:], cos_sb[:, s0 : s0 + QW]
                            )
                            nc.vector.tensor_add(
                                qk[i][:, s0 : s0 + QW], t1[:], t2[:]
                            )
                        else:
                            nc.scalar.activation(
                                vT_sb[:, s0 : s0 + QW], ps[i][:], AF.Copy
                            )
                            for sbl in range(QW // 128):
                                sb = q * (QW // 128) + sbl
                                psvt = psv.tile([128, 128], mm_dt, tag="psv")
                                nc.tensor.transpose(
                                    psvt[:],
                                    vT_sb[:, sb * 128 : (sb + 1) * 128],
                                    ident[:],
                                )
                                nc.scalar.activation(
                                    v_sb[:, sb * 128 : (sb + 1) * 128], psvt[:],
                                    AF.Copy,
                                )

            # ---- Phase 2 + 3 interleaved per query block ----
            with (
                contextlib.nullcontext() if phases >= 2 else _Skip(),
                tc.tile_pool(name="attn", bufs=1) as ap,
                tc.tile_pool(name="epool", bufs=8) as ep,
                tc.tile_pool(name="accp", bufs=2) as accp,
                tc.tile_pool(name="small", bufs=4) as sp,
                tc.tile_pool(name="osb", bufs=2) as op,
                tc.tile_pool(name="pss", bufs=3, space="PSUM") as pss,
                tc.tile_pool(name="pspv", bufs=1, space="PSUM") as pspv,
                tc.tile_pool(name="psden", bufs=1, space="PSUM") as psden,
                tc.tile_pool(name="pso", bufs=2, space="PSUM") as pso,
            ):
                attnT = [
                    ap.tile([128, S], mm_dt, name=f"at{h}", tag=f"at{h}")
                    for h in range(4)
                ]

                kT = qk[4]

                # o-projection is deferred one query block and interleaved
                # into the next block's chain loops: PE does o-proj matmuls
                # while ACT grinds the exps (they were serializing before).
                pending = []
                osb_box = [None]

                def oproj_unit(sb, n):
                    def emit():
                        if n == 0:
                            osb_box[0] = op.tile(
                                [128, H], mm_dt, name=f"osb{sb}", tag="osb"
                            )
                        osb = osb_box[0]
                        pst = pso.tile([128, NQ], F32, tag="po")
                        for kb in range(4):
                            nc.tensor.matmul(
                                pst[:],
                                lhsT=attnT[kb][:, sb * 128 : (sb + 1) * 128],
                                rhs=wo_sb[:, kb * H + n * NQ : kb * H + (n + 1) * NQ],
                                start=(kb == 0),
                                stop=(kb == 3),
                            )
                        nc.vector.tensor_copy(osb[:, n * NQ : (n + 1) * NQ], pst[:])
                        if n == 3:
                            nc.sync.dma_start(
                                out=out[sb * 128 : (sb + 1) * 128, :], in_=osb[:]
                            )
                    return emit

                def flush_oproj(k):
                    for _ in range(k):
                        if pending:
                            pending.pop(0)()

                for qb in range(S // NQ):
                    q0 = qb * NQ
                    nj = 4 * qb + 4
                    # process heads in pairs with interleaved j-loops so the
                    # PE always has an independent chain's matmul ready while
                    # the other chain waits on exp
                    for pair in range(2):
                        hs2 = (2 * pair, 2 * pair + 1)
                        pv = {}
                        acc = {}
                        for h in hs2:
                            pv[h] = pspv.tile([128, NQ], F32, name=f"pv{h}", tag=f"pv{h % 2}")
                            acc[h] = accp.tile([128, NQ], mm_dt, name=f"acc{h}", tag=f"acc{h % 2}")
                        for j in range(nj):
                          r4 = j - 4 * qb
                          # diagonal blocks: columns sq < r4*128 are fully
                          # masked -> narrow the whole j-chain to [off:NQ)
                          off = max(0, r4) * 128
                          for h in hs2:
                            qT = qk[h]
                            sps = pss.tile([128, NQ], F32, tag="sc")
                            nc.tensor.matmul(
                                sps[:, off:NQ],
                                lhsT=kT[:, j * 128 : (j + 1) * 128],
                                rhs=qT[:, q0 + off : q0 + NQ],
                                start=True,
                                stop=True,
                            )
                            e = ep.tile([128, NQ], mm_dt, tag="e")
                            nc.scalar.activation(
                                e[:, off:NQ], sps[:, off:NQ], AF.Exp, scale=SCALE
                            )
                            if r4 >= 0:
                                # causal mask as post-exp 0/1 multiply on the
                                # 128 diagonal cols -- keeps DVE off the
                                # scores->exp critical path (ACT-bound)
                                nc.vector.tensor_mul(
                                    e[:, off : off + 128],
                                    e[:, off : off + 128],
                                    tri_sb[:],
                                )
                            nc.tensor.matmul(
                                pv[h][:, off:NQ],
                                lhsT=v_sb[:, j * 128 : (j + 1) * 128],
                                rhs=e[:, off:NQ],
                                start=(j == 0),
                                stop=(j == nj - 1),
                            )
                            # softmax denominator: running sum of e on DVE
                            if j == 0:
                                nc.vector.tensor_copy(acc[h][:], e[:])
                            else:
                                nc.vector.tensor_add(
                                    acc[h][:, off:NQ], acc[h][:, off:NQ],
                                    e[:, off:NQ],
                                )
                          # one deferred o-proj chain per j-step keeps the
                          # PE busy while ACT runs this step's exps
                          flush_oproj(1)
                        for h in hs2:
                            # partition-reduce acc on the PE (ones-matmul:
                            # one N=512 pass per chain, ~150ns)
                            den = psden.tile([1, NQ], F32, tag="den")
                            nc.tensor.matmul(
                                den[:], lhsT=ones_r[:], rhs=acc[h][:],
                                start=True, stop=True,
                            )
                            rec = sp.tile([1, NQ], F32, tag="rec")
                            nc.vector.reciprocal(rec[:], den[:])
                            bcs = sp.tile([128, NQ], F32, tag="bcs")
                            nc.gpsimd.partition_broadcast(bcs[:], rec[:])
                            nc.vector.tensor_mul(
                                attnT[h][:, q0 : q0 + NQ], pv[h][:], bcs[:]
                            )
                    # queue this query block's o-projection for interleaved
                    # emission during the next block's chains
                    if phases >= 3:
                        for sbl in range(NQ // 128):
                            sb = qb * 4 + sbl
                            for n in range(H // NQ):
                                pending.append(oproj_unit(sb, n))
                flush_oproj(len(pending))

    nc.compile()
    return nc


def _prep_inputs(hidden_states, cos, sin, w_qkv, w_o):
    """Build the 8 per-core input maps (host-side shard + transpose, bf16)."""
    hidden_states = np.asarray(hidden_states, dtype=np.float32)
    cos = np.asarray(cos, dtype=np.float32)
    sin = np.asarray(sin, dtype=np.float32)
    w_qkv = np.asarray(w_qkv, dtype=np.float32)
    w_o = np.asarray(w_o, dtype=np.float32)

    cosT = np.ascontiguousarray(cos.T).astype(NPBF16)
    # sin table half-swapped with rotate_half sign folded in:
    #   rows 0:64  hold  sin.T[64:128]  (multiplies raw[0:64] -> t1[64:128])
    #   rows 64:128 hold -sin.T[0:64]   (multiplies raw[64:128] -> t1[0:64])
    sinTf = np.ascontiguousarray(sin.T)
    sinT = np.concatenate([sinTf[64:128], -sinTf[0:64]]).astype(NPBF16)

    # tri[sk, sq] = 1 if sk <= sq else 0  (post-exp diagonal-block mask)
    sk = np.arange(128)[:, None]
    sq = np.arange(128)[None, :]
    tri = np.where(sk <= sq, 1.0, 0.0).astype(NPBF16)

    # hTp[p, q, kt, s'] = hidden[b].T[kt*128+p, q*512+s']
    hTp = [
        np.ascontiguousarray(
            hidden_states[b].T.reshape(KT, 128, 4, 512).transpose(1, 2, 0, 3)
        ).astype(NPBF16)
        for b in range(2)
    ]

    in_maps = []
    for c in range(8):
        b, g = divmod(c, 4)
        W6 = np.stack(
            [w_qkv[(4 * g + i) * 128 : (4 * g + i + 1) * 128] for i in range(4)]
            + [w_qkv[(16 + g) * 128 : (17 + g) * 128]]
            + [w_qkv[(20 + g) * 128 : (21 + g) * 128]]
        )  # [6 i, 128 m, 2048 h]
        # wqk_pack[p, kt, i, m] = W6[i, m, kt*128+p]
        wqk_pack = np.ascontiguousarray(
            W6.transpose(2, 0, 1).reshape(KT, 128, 6, 128).transpose(1, 0, 2, 3)
        ).astype(NPBF16)
        # wo_pack[d, kb, o] = w_o[o, (4g+kb)*128+d]
        wo_pack = np.ascontiguousarray(
            w_o[:, 4 * g * 128 : (4 * g + 4) * 128]
            .T.reshape(4, 128, H)
            .transpose(1, 0, 2)
        ).astype(NPBF16)
        in_maps.append(
            dict(
                hTp=hTp[b],
                wqk=wqk_pack,
                cosT=cosT,
                sinT=sinT,
                tri=tri,
                wo=wo_pack,
            )
        )
    return in_maps


def run(hidden_states, cos, sin, w_qkv, w_o, trace=False, **trace_kwargs):
    if "nc" not in _CACHED:
        _CACHED["nc"] = build_nc()
    nc = _CACHED["nc"]
    in_maps = _prep_inputs(hidden_states, cos, sin, w_qkv, w_o)
    res = run_bass_kernel_spmd(
        nc, in_maps, core_ids=list(range(8)), trace=trace, **trace_kwargs
    )
    outs = [res.results[c]["out"].astype(np.float32) for c in range(8)]
    full = np.stack(
        [
            outs[0] + outs[1] + outs[2] + outs[3],
            outs[4] + outs[5] + outs[6] + outs[7],
        ]
    ).astype(np.float32)
    return full, res


def kernel(hidden_states, cos, sin, w_qkv, w_o):
    full, _ = run(hidden_states, cos, sin, w_qkv, w_o, trace=False)
    return full


# revision 42
# speedup vs baseline: 1.3671x; 1.1731x over previous
"""Trainium2 Bass kernel for GQA attention block (B=2, S=2048, H=2048,
16 q-heads / 4 kv-heads, head_dim=128, RoPE, causal) on 8 NeuronCores.

Sharding: core c -> batch b = c // 4, kv-group g = c % 4
  (q heads 4g..4g+3, kv head g).  Each core computes its batch's
  attention for its 4 query heads plus the partial output projection
  over its 512 hidden columns of w_o; host sums the 4 partials per batch.

v3: bf16 end-to-end (halves HBM traffic + DVE cost; fp32 PSUM accumulate
keeps rel err ~1e-3).  QKV projection iterates kt-outer across 6
parallel PSUM chains so each weight/activation slab is consumed as it
lands (PE starts ~2us after launch).  DMA loads are batched and split
across the SP and Activation HWDGE queues.  RoPE rotate-half is done
with partition-offset DVE ops (no SBUF->SBUF DMAs).  The softmax
denominator is accumulated on DVE (e-tile running sum) and reduced
across partitions with one GpSimd partition_all_reduce per (qb, head),
freeing ~29us of PE ones-matmul time.  Causal mask add narrowed to the
128-wide diagonal sub-block.

On-chip layouts (per core):
  qT/kT    [head_dim=128 part, S free] bf16  (projection emits transposed)
  v        [S part-blocks, head_dim] bf16    (PE transpose of vT; PV lhsT)
  scoresT  [sk part, sq free] f32 PSUM -> exp -> bf16 e
  PV accumulates out^T [d, sq] in PSUM over j
  o-proj emits out[s, o] bf16; host upcasts + sums 4 partials per batch
"""

import contextlib
import math
import numpy as np

import concourse.bacc as bacc
import concourse.bass_isa as bass_isa
import concourse.mybir as mybir
import concourse.tile as tile
from concourse.bass_utils import run_bass_kernel_spmd
from concourse.masks import make_identity

F32 = mybir.dt.float32
BF16 = mybir.dt.bfloat16
AF = mybir.ActivationFunctionType
NPBF16 = mybir.dt.np(BF16)

S = 2048
H = 2048
D = 128            # head dim
KT = 16            # contraction tiles over hidden (2048/128)
NQ = 512           # query block width in attention
NUM_Q_LOCAL = 4    # q heads per core
SCALE = 1.0 / math.sqrt(D)
NEG = -1.0e9

_CACHED = {}


class _SkipExc(Exception):
    pass


class _Skip:
    """Context manager that skips its with-body entirely."""

    def __enter__(self):
        import sys
        import inspect
        self._tr = sys.gettrace()
        sys.settrace(lambda *a, **k: None)
        frame = inspect.currentframe().f_back
        frame.f_trace = self._trace
        return self

    def _trace(self, frame, event, arg):
        raise _SkipExc

    def __exit__(self, exc_type, exc, tb):
        import sys
        sys.settrace(self._tr)
        return exc_type is _SkipExc


def build_nc(mm_dt=BF16, loop_n=None, phases=3):
    nc = bacc.Bacc(None, target_bir_lowering=False)
    # host-packed layouts (see _prep_inputs):
    #   hTp  [128 p, 16 kt, 2048 s]   hidden[b].T, p = h % 128, kt = h // 128
    #   wqk  [128 p, 16 kt, 6 i, 128 m]
    #   wo   [128 d, 4 kb, 2048 o]
    hTp = nc.dram_tensor("hTp", [128, 4, KT, 512], mm_dt, kind="ExternalInput")
    wqk = nc.dram_tensor("wqk", [128, KT, 6, 128], mm_dt, kind="ExternalInput")
    cosT = nc.dram_tensor("cosT", [D, S], mm_dt, kind="ExternalInput")
    sinT = nc.dram_tensor("sinT", [D, S], mm_dt, kind="ExternalInput")
    tri = nc.dram_tensor("tri", [128, 128], mm_dt, kind="ExternalInput")
    wo = nc.dram_tensor("wo", [128, 4, H], mm_dt, kind="ExternalInput")
    out = nc.dram_tensor("out", [S, H], mm_dt, kind="ExternalOutput")

    wqk_flat = wqk.reshape([128, KT * 6 * 128])
    wo_flat = wo.reshape([128, 4 * H])

    with tile.TileContext(nc) as tc:
        with tc.tile_pool(name="persist", bufs=1) as pp:
          with (tc.For_i(0, loop_n, 1) if loop_n else contextlib.nullcontext()):
            # ---- persistent tiles (live across phases) ----
            qk = [pp.tile([128, S], mm_dt, name=f"qk{i}", tag=f"qk{i}") for i in range(5)]
            v_sb = pp.tile([128, S], mm_dt, tag="v")
            ones_r = pp.tile([128, 1], mm_dt, tag="onesr")
            cos_sb = pp.tile([128, S], mm_dt, tag="cos")
            sin_sb = pp.tile([128, S], mm_dt, tag="sin")
            ident = pp.tile([128, 128], mm_dt, tag="ident")
            tri_sb = pp.tile([128, 128], mm_dt, tag="tri")
            w_sb = pp.tile([128, KT * 6 * 128], mm_dt, tag="wsb")
            wo_sb = pp.tile([128, 4 * H], mm_dt, tag="wosb")

            # ---- Phase 1: fused QKV projection, kt-outer over 6 PSUM chains
            # + RoPE and v-transpose per quarter ----
            QW = 512
            NQT = S // QW
            with (
                tc.tile_pool(name="ht", bufs=2) as htp,
                tc.tile_pool(name="vtp", bufs=1) as vtp,
                tc.tile_pool(name="rope", bufs=3) as rp,
                tc.tile_pool(name="psq", bufs=1, space="PSUM") as psq,
                tc.tile_pool(name="psv", bufs=2, space="PSUM") as psv,
            ):
                vT_sb = vtp.tile([128, S], mm_dt, tag="vT")
                ht = [None] * NQT

                def load_ht(q, split=False):
                    # per-partition contiguous runs: 4KB (split groups) or
                    # 16KB (whole quarter)
                    ht[q] = htp.tile(
                        [128, KT, QW], mm_dt, name=f"htq{q}", tag=f"ht{q % 2}"
                    )
                    if split:
                        for g4 in range(4):
                            nc.sync.dma_start(
                                out=ht[q][:, g4 * 4 : (g4 + 1) * 4, :],
                                in_=hTp[:, q, g4 * 4 : (g4 + 1) * 4, :],
                            )
                    else:
                        nc.sync.dma_start(out=ht[q][:], in_=hTp[:, q])

                # first weight slab + first ht group lead their rings so the
                # first matmul starts ~2us in; graduated slab sizes after
                ht[0] = htp.tile([128, KT, QW], mm_dt, name="htq0", tag="ht0")
                nc.scalar.dma_start(
                    out=w_sb[:, 0:768], in_=wqk_flat[:, 0:768]
                )
                nc.sync.dma_start(out=ht[0][:, 0:4, :], in_=hTp[:, 0, 0:4, :])
                nc.scalar.dma_start(
                    out=w_sb[:, 768 : 4 * 768], in_=wqk_flat[:, 768 : 4 * 768]
                )
                nc.sync.dma_start(out=ht[0][:, 4:16, :], in_=hTp[:, 0, 4:16, :])
                for half in range(2):
                    sl = slice((4 + 6 * half) * 768, (10 + 6 * half) * 768)
                    nc.scalar.dma_start(out=w_sb[:, sl], in_=wqk_flat[:, sl])
                nc.scalar.dma_start(out=cos_sb[:], in_=cosT[:])
                nc.scalar.dma_start(out=sin_sb[:], in_=sinT[:])
                nc.scalar.dma_start(out=tri_sb[:], in_=tri[:])
                nc.scalar.dma_start(out=wo_sb[:], in_=wo_flat[:])
                nc.vector.memset(ones_r[:], 1.0)
                make_identity(nc, ident[:])

                for q in range(NQT):
                    s0 = q * QW
                    if q + 1 < NQT:
                        load_ht(q + 1)
                    ps = [
                        psq.tile([128, QW], F32, name=f"ps{i}", tag=f"psq{i}")
                        for i in range(6)
                    ]
                    for kt in range(KT):
                        for i in range(6):
                            nc.tensor.matmul(
                                ps[i][:],
                                lhsT=w_sb[:, (kt * 6 + i) * 128 : (kt * 6 + i + 1) * 128],
                                rhs=ht[q][:, kt, :],
                                start=(kt == 0),
                                stop=(kt == KT - 1),
                            )
                    for i in range(6):
                        if i < 5:
                            # RoPE: qk = raw*cos + rot_half(raw)*sin
                            # (sin rows 0:64 pre-negated host-side)
                            # SB+SB tensor ops need equal INPUT base
                            # partitions; sin is stored half-swapped
                            # host-side so the rotate-half muls read raw and
                            # sin at the same base and only the OUTPUT is
                            # partition-shifted.
                            raw = rp.tile([128, QW], mm_dt, tag="raw")
                            if i % 2 == 0:
                                nc.scalar.activation(raw[:], ps[i][:], AF.Copy)
                            else:
                                nc.vector.tensor_copy(raw[:], ps[i][:])
                            t1 = rp.tile([128, QW], mm_dt, tag="t1")
                            t2 = rp.tile([128, QW], mm_dt, tag="t2")
                            nc.vector.tensor_mul(
                                t1[64:128, :], raw[0:64, :], sin_sb[0:64, s0 : s0 + QW]
                            )
                            nc.vector.tensor_mul(
                                t1[0:64, :], raw[64:128, :], sin_sb[64:128, s0 : s0 + QW]
                            )
                            nc.vector.tensor_mul(
                                t2[:], raw[:], cos_sb[:, s0 : s0 + QW]
                            )
                            nc.vector.tensor_add(
                                qk[i][:, s0 : s0 + QW], t1[:], t2[:]
                            )
                        else:
                            nc.scalar.activation(
                                vT_sb[:, s0 : s0 + QW], ps[i][:], AF.Copy
                            )
                            for sbl in range(QW // 128):
                                sb = q * (QW // 128) + sbl
                                psvt = psv.tile([128, 128], mm_dt, tag="psv")
                                nc.tensor.transpose(
                                    psvt[:],
                                    vT_sb[:, sb * 128 : (sb + 1) * 128],
                                    ident[:],
                                )
                                nc.scalar.activation(
                                    v_sb[:, sb * 128 : (sb + 1) * 128], psvt[:],
                                    AF.Copy,
                                )

            # ---- Phase 2 + 3 interleaved per query block ----
            with (
                contextlib.nullcontext() if phases >= 2 else _Skip(),
                tc.tile_pool(name="attn", bufs=1) as ap,
                tc.tile_pool(name="epool", bufs=8) as ep,
                tc.tile_pool(name="accp", bufs=1) as accp,
                tc.tile_pool(name="small", bufs=2) as sp,
                tc.tile_pool(name="osb", bufs=2) as op,
                tc.tile_pool(name="pss", bufs=2, space="PSUM") as pss,
                tc.tile_pool(name="pspv", bufs=1, space="PSUM") as pspv,
                tc.tile_pool(name="pso", bufs=2, space="PSUM") as pso,
            ):
                attnT = [
                    ap.tile([128, S], mm_dt, name=f"at{h}", tag=f"at{h}")
                    for h in range(4)
                ]

                kT = qk[4]

                # o-projection is deferred one query block and interleaved
                # into the next block's chain loops: PE does o-proj matmuls
                # while ACT grinds the exps (they were serializing before).
                pending = []
                osb_box = [None]

                def oproj_unit(sb, n):
                    def emit():
                        if n == 0:
                            osb_box[0] = op.tile(
                                [128, H], mm_dt, name=f"osb{sb}", tag="osb"
                            )
                        osb = osb_box[0]
                        pst = pso.tile([128, NQ], F32, tag="po")
                        for kb in range(4):
                            nc.tensor.matmul(
                                pst[:],
                                lhsT=attnT[kb][:, sb * 128 : (sb + 1) * 128],
                                rhs=wo_sb[:, kb * H + n * NQ : kb * H + (n + 1) * NQ],
                                start=(kb == 0),
                                stop=(kb == 3),
                            )
                        nc.vector.tensor_copy(osb[:, n * NQ : (n + 1) * NQ], pst[:])
                        if n == 3:
                            nc.sync.dma_start(
                                out=out[sb * 128 : (sb + 1) * 128, :], in_=osb[:]
                            )
                    return emit

                def flush_oproj(k):
                    for _ in range(k):
                        if pending:
                            pending.pop(0)()

                for qb in range(S // NQ):
                    q0 = qb * NQ
                    nj = 4 * qb + 4
                    # all 4 heads run their j-loops in lockstep: 4
                    # independent chains in flight hide the per-step
                    # mm->exp->mm latency (2-chain version was latency-bound
                    # at ~2.1us per step for ~1.2us of engine work)
                    pv = {}
                    acc = {}
                    for h in range(NUM_Q_LOCAL):
                        pv[h] = pspv.tile([128, NQ], F32, name=f"pv{h}", tag=f"pv{h}")
                        acc[h] = accp.tile([128, NQ], mm_dt, name=f"acc{h}", tag=f"acc{h}")
                    for j in range(nj):
                        r4 = j - 4 * qb
                        # diagonal blocks: columns sq < r4*128 are fully
                        # masked -> narrow the whole j-chain to [off:NQ)
                        off = max(0, r4) * 128
                        for h in range(NUM_Q_LOCAL):
                            qT = qk[h]
                            sps = pss.tile([128, NQ], F32, tag="sc")
                            nc.tensor.matmul(
                                sps[:, off:NQ],
                                lhsT=kT[:, j * 128 : (j + 1) * 128],
                                rhs=qT[:, q0 + off : q0 + NQ],
                                start=True,
                                stop=True,
                            )
                            e = ep.tile([128, NQ], mm_dt, tag="e")
                            nc.scalar.activation(
                                e[:, off:NQ], sps[:, off:NQ], AF.Exp, scale=SCALE
                            )
                            if r4 >= 0:
                                # causal mask as post-exp 0/1 multiply on the
                                # 128 diagonal cols -- keeps DVE off the
                                # scores->exp critical path (ACT-bound)
                                nc.vector.tensor_mul(
                                    e[:, off : off + 128],
                                    e[:, off : off + 128],
                                    tri_sb[:],
                                )
                            nc.tensor.matmul(
                                pv[h][:, off:NQ],
                                lhsT=v_sb[:, j * 128 : (j + 1) * 128],
                                rhs=e[:, off:NQ],
                                start=(j == 0),
                                stop=(j == nj - 1),
                            )
                            # softmax denominator: running sum of e on DVE
                            if j == 0:
                                nc.vector.tensor_copy(acc[h][:], e[:])
                            else:
                                nc.vector.tensor_add(
                                    acc[h][:, off:NQ], acc[h][:, off:NQ],
                                    e[:, off:NQ],
                                )
                        # two deferred o-proj chains per j-step keep the PE
                        # busy while ACT runs this step's exps
                        flush_oproj(2)
                    for h in range(NUM_Q_LOCAL):
                        # partition-reduce acc on GpSimd (output arrives
                        # already broadcast across partitions)
                        den = sp.tile([128, NQ], F32, tag="den")
                        nc.gpsimd.partition_all_reduce(
                            den[:], acc[h][:], channels=128,
                            reduce_op=bass_isa.ReduceOp.add,
                        )
                        rec = sp.tile([128, NQ], F32, tag="rec")
                        nc.vector.reciprocal(rec[:], den[:])
                        nc.vector.tensor_mul(
                            attnT[h][:, q0 : q0 + NQ], pv[h][:], rec[:]
                        )
                    # queue this query block's o-projection for interleaved
                    # emission during the next block's chains
                    if phases >= 3:
                        for sbl in range(NQ // 128):
                            sb = qb * 4 + sbl
                            for n in range(H // NQ):
                                pending.append(oproj_unit(sb, n))
                flush_oproj(len(pending))

    nc.compile()
    return nc


def _prep_inputs(hidden_states, cos, sin, w_qkv, w_o):
    """Build the 8 per-core input maps (host-side shard + transpose, bf16)."""
    hidden_states = np.asarray(hidden_states, dtype=np.float32)
    cos = np.asarray(cos, dtype=np.float32)
    sin = np.asarray(sin, dtype=np.float32)
    w_qkv = np.asarray(w_qkv, dtype=np.float32)
    w_o = np.asarray(w_o, dtype=np.float32)

    cosT = np.ascontiguousarray(cos.T).astype(NPBF16)
    # sin table half-swapped with rotate_half sign folded in:
    #   rows 0:64  hold  sin.T[64:128]  (multiplies raw[0:64] -> t1[64:128])
    #   rows 64:128 hold -sin.T[0:64]   (multiplies raw[64:128] -> t1[0:64])
    sinTf = np.ascontiguousarray(sin.T)
    sinT = np.concatenate([sinTf[64:128], -sinTf[0:64]]).astype(NPBF16)

    # tri[sk, sq] = 1 if sk <= sq else 0  (post-exp diagonal-block mask)
    sk = np.arange(128)[:, None]
    sq = np.arange(128)[None, :]
    tri = np.where(sk <= sq, 1.0, 0.0).astype(NPBF16)

    # hTp[p, q, kt, s'] = hidden[b].T[kt*128+p, q*512+s']
    hTp = [
        np.ascontiguousarray(
            hidden_states[b].T.reshape(KT, 128, 4, 512).transpose(1, 2, 0, 3)
        ).astype(NPBF16)
        for b in range(2)
    ]

    in_maps = []
    for c in range(8):
        b, g = divmod(c, 4)
        W6 = np.stack(
            [w_qkv[(4 * g + i) * 128 : (4 * g + i + 1) * 128] for i in range(4)]
            + [w_qkv[(16 + g) * 128 : (17 + g) * 128]]
            + [w_qkv[(20 + g) * 128 : (21 + g) * 128]]
        )  # [6 i, 128 m, 2048 h]
        # wqk_pack[p, kt, i, m] = W6[i, m, kt*128+p]
        wqk_pack = np.ascontiguousarray(
            W6.transpose(2, 0, 1).reshape(KT, 128, 6, 128).transpose(1, 0, 2, 3)
        ).astype(NPBF16)
        # wo_pack[d, kb, o] = w_o[o, (4g+kb)*128+d]
        wo_pack = np.ascontiguousarray(
            w_o[:, 4 * g * 128 : (4 * g + 4) * 128]
            .T.reshape(4, 128, H)
            .transpose(1, 0, 2)
        ).astype(NPBF16)
        in_maps.append(
            dict(
                hTp=hTp[b],
                wqk=wqk_pack,
                cosT=cosT,
                sinT=sinT,
                tri=tri,
                wo=wo_pack,
            )
        )
    return in_maps


def run(hidden_states, cos, sin, w_qkv, w_o, trace=False, **trace_kwargs):
    if "nc" not in _CACHED:
        _CACHED["nc"] = build_nc()
    nc = _CACHED["nc"]
    in_maps = _prep_inputs(hidden_states, cos, sin, w_qkv, w_o)
    res = run_bass_kernel_spmd(
        nc, in_maps, core_ids=list(range(8)), trace=trace, **trace_kwargs
    )
    outs = [res.results[c]["out"].astype(np.float32) for c in range(8)]
    full = np.stack(
        [
            outs[0] + outs[1] + outs[2] + outs[3],
            outs[4] + outs[5] + outs[6] + outs[7],
        ]
    ).astype(np.float32)
    return full, res


def kernel(hidden_states, cos, sin, w_qkv, w_o):
    full, _ = run(hidden_states, cos, sin, w_qkv, w_o, trace=False)
    return full
